# revision 20
# baseline (speedup 1.0000x reference)
"""GCN (3-layer, PyG GCNConv-style) Trainium2 Bass kernel, 8 NeuronCores.

Strategy:
  - Nodes row-sharded: core p owns rows [p*6250, (p+1)*6250).
  - Per layer: h' = (dinv*act) @ W computed shardwise (PE; dinv folded into a
    scaled-transpose matmul), bf16 table exchanged via TWO sub-AllGathers
    split at local row ASPLIT so A-half gathers start before AG-B lands.
  - Self-loop contributions are NOT gathered: hp tiles are kept in SBUF and
    injected into each tile's PSUM accumulation via one identity matmul.
  - Non-loop edges partitioned by dst owner, grouped by dst tile (128 dsts),
    padded to 128-slot blocks; messages fetched with chunked SWDGE dma_gather
    (int16 indices into the two A/B tables), spread across 4 SWDGE queues so
    descriptor generation runs on all four Q7 core pairs concurrently.
  - Segmented sum per 128-message block via PE matmul with 0/1 selection
    matrices built ON-CHIP by DVE (iota vs col-index is_equal compare)
    accumulated in PSUM (no scatter-add races).
  - Post: out = relu(dinv*agg + b); next-layer z = dinv*out. Final layer:
    sigmoid(act3 @ Wout + bout).
"""

import numpy as np

N = 50000
E_F = 128          # feature dim
CORES = 8
NP = 6250          # nodes per core
NT = 49            # 128-row output tiles per core (6272 slots, 22 pad)
TP = 128
SLICE = 6304       # table rows per core (6250 + 54 zeros)
# local-row groups (tile-aligned) -> one exchange table + gather stream each,
# so each sub-AllGather fires as soon as its source tiles finalize
GRPS = [(0, 2560), (2560, 4480), (4480, 6304)]
NG = len(GRPS)
GROWS = [8 * (b - a) for a, b in GRPS]
CHB = 8            # gather-chunk size in 128-slot blocks (1024 idxs)
SGRP = 8           # S-build group size in blocks
PAD_IDX = 0        # pad slots gather row 0; their S rows are all-zero
PAD_COL = 255      # out-of-range col -> is_equal never fires -> zero S row

_compiled = None   # (nc, meta) cache


def _wrap_idxs(idx):
    """idx j -> partition j%16, column j//16; replicated to 128 partitions."""
    assert idx.size % 16 == 0
    cols = idx.size // 16
    w = idx.reshape(cols, 16).T.astype(np.int16)
    return np.tile(w, (8, 1))


def _grp_rel(g):
    """Global node id -> (group, relative row) under the local-row groups."""
    q, r = g // NP, g % NP
    grp = np.zeros_like(g)
    rel = np.zeros_like(g)
    for gi, (a, b) in enumerate(GRPS):
        m = (r >= a) & (r < b)
        grp[m] = gi
        rel[m] = q[m] * (b - a) + (r[m] - a)
    return grp, rel


def _preprocess(edge_index):
    """Build per-core gather index streams + per-slot dst-column indices.

    Self-loops are excluded: they are injected on-chip from the local hp
    tiles. Returns (meta, per_core).
    """
    src = np.asarray(edge_index[0]).astype(np.int64)
    dst = np.asarray(edge_index[1]).astype(np.int64)

    half, rel = _grp_rel(src)
    owner = dst // NP
    dloc = dst - owner * NP        # 0..6249
    tile = dloc // TP
    col = dloc - tile * TP         # 0..127

    # counts[p, t, h]
    counts = np.zeros((CORES, NT, NG), np.int64)
    np.add.at(counts, (owner, tile, half), 1)
    blocks = (np.ceil(counts.max(axis=0) / TP).astype(np.int64))  # [NT, NG]
    blocks = np.maximum(blocks, 1)

    per_core = []
    TBG = [int(blocks[:, h].sum()) for h in range(NG)]
    TB = sum(TBG)
    slots = TB * TP

    # block schedule (same for all cores): list of (tile, grp)
    sched = []
    for h in range(NG):
        for t in range(NT):
            sched += [(t, h)] * int(blocks[t, h])

    # chunk split: group-major, chunks of <= CHB blocks, not crossing
    # group boundaries
    chunks = []  # (grp, start_block_global, nblocks)
    base = 0
    for h in range(NG):
        off = 0
        while off < TBG[h]:
            n = min(CHB, TBG[h] - off)
            chunks.append((h, base + off, n))
            off += n
        base += TBG[h]

    # group key ordering: grp-major then tile (matches sched/stream layout)
    nblk_flat = np.array(
        [int(blocks[t, h]) for h in range(NG) for t in range(NT)], np.int64
    )
    base_slot_flat = np.concatenate([[0], np.cumsum(nblk_flat * TP)[:-1]])
    blk_base_flat = np.concatenate([[0], np.cumsum(nblk_flat)[:-1]])

    import ml_dtypes
    for p in range(CORES):
        sel = owner == p
        rel_p, half_p = rel[sel], half[sel]
        tile_p, col_p = tile[sel], col[sel]
        key = half_p * NT + tile_p
        order = np.argsort(key, kind="stable")
        ks = key[order]
        # rank within group
        starts = np.searchsorted(ks, np.arange(NG * NT))
        rank = np.arange(ks.size) - starts[ks]
        slot = base_slot_flat[ks] + rank
        gidx = np.full(slots, PAD_IDX, np.int64)
        gidx[slot] = rel_p[order]
        # per-slot dst column: [TB, 128] -> transposed [128, TB] bf16
        colv = np.full((TB, TP), PAD_COL, np.float32)
        colv[blk_base_flat[ks] + rank // TP, rank % TP] = col_p[order]
        per_core.append(
            {
                "gidx": _wrap_idxs(gidx.astype(np.int16)),
                "col": colv.T.astype(ml_dtypes.bfloat16).copy(),
            }
        )

    meta = {
        "blocks": blocks,
        "sched": sched,
        "chunks": chunks,
        "TB": TB,
        "TBG": TBG,
        "slots": slots,
    }
    return meta, per_core


def _build(meta):
    import concourse.bacc as bacc
    import concourse.bass as bass
    import concourse.mybir as mybir
    import concourse.tile as tile

    f32 = mybir.dt.float32
    bf16 = mybir.dt.bfloat16
    nc = bacc.Bacc("TRN2", target_bir_lowering=False, num_swdge_queues=4)

    TB, slots = meta["TB"], meta["slots"]
    icols = slots // 16

    x_in = nc.dram_tensor("x_in", [NT * TP, E_F], f32, kind="ExternalInput")
    dinv_in = nc.dram_tensor("dinv_in", [TP, NT], f32, kind="ExternalInput")
    Ws = [
        nc.dram_tensor(f"W{i}", [E_F, E_F], f32, kind="ExternalInput")
        for i in (1, 2, 3)
    ]
    Bs = [
        nc.dram_tensor(f"b{i}", [1, E_F], f32, kind="ExternalInput")
        for i in (1, 2, 3)
    ]
    dg_in = nc.dram_tensor("dg_in", [TP, NT * TP], bf16, kind="ExternalInput")
    c_in = nc.dram_tensor("c_in", [1, NT * TP], f32, kind="ExternalInput")
    Wout = nc.dram_tensor("Wout", [E_F, 1], f32, kind="ExternalInput")
    Woutb = nc.dram_tensor("Woutb", [E_F, 1], bf16, kind="ExternalInput")
    bout = nc.dram_tensor("bout", [TP, 1], f32, kind="ExternalInput")
    ident = nc.dram_tensor("ident", [TP, TP], f32, kind="ExternalInput")
    identb = nc.dram_tensor("identb", [TP, TP], bf16, kind="ExternalInput")
    iota_in = nc.dram_tensor("iota_in", [TP, TP], bf16, kind="ExternalInput")
    gidx_in = nc.dram_tensor("gidx_in", [128, icols], mybir.dt.int16, kind="ExternalInput")
    col_in = nc.dram_tensor("col_in", [TP, TB], bf16, kind="ExternalInput")
    y_out = nc.dram_tensor("y_out", [TP, NT], f32, kind="ExternalOutput")

    bounce = nc.dram_tensor("bounce", [SLICE, E_F], bf16)
    tables = [
        [
            nc.dram_tensor(f"table{g}_{l}", [GROWS[g], E_F], bf16, addr_space="Shared")
            for g in range(NG)
        ]
        for l in range(3)
    ]

    with tile.TileContext(nc) as tc:
        with (
            tc.tile_pool(name="const", bufs=1) as cpool,
            tc.tile_pool(name="big", bufs=1) as zpool,
            tc.tile_pool(name="msg", bufs=24) as mpool,
            tc.tile_pool(name="sld", bufs=8) as spool,
            tc.tile_pool(name="work", bufs=3) as wpool,
            tc.tile_pool(name="pst", bufs=2, space="PSUM") as pst,
            tc.tile_pool(name="psh", bufs=2, space="PSUM") as psh,
            tc.tile_pool(name="pss", bufs=4, space="PSUM") as pss,
        ):
            # constants
            w_sb = [cpool.tile([E_F, E_F], f32, tag=f"w{i}", name=f"w_sb{i}") for i in range(3)]
            for i in range(3):
                nc.sync.dma_start(w_sb[i][:], Ws[i][:])
            b_sb = [cpool.tile([1, E_F], f32, tag=f"b{i}", name=f"b_sb{i}") for i in range(3)]
            dg_sb = cpool.tile([TP, NT * TP], bf16, tag="dg")
            nc.sync.dma_start(dg_sb[:], dg_in[:])
            c_sb = cpool.tile([1, NT * TP], f32, tag="cc")
            nc.sync.dma_start(c_sb[:], c_in[:])
            for i in range(3):
                nc.sync.dma_start(b_sb[i][:], Bs[i][:])
            woutb_sb = cpool.tile([E_F, 1], bf16, tag="wo")
            nc.sync.dma_start(woutb_sb[:], Woutb[:])
            bout_sb = cpool.tile([TP, 1], f32, tag="bo")
            nc.sync.dma_start(bout_sb[:], bout[:])
            id_sb = cpool.tile([TP, TP], f32, tag="id")
            nc.sync.dma_start(id_sb[:], ident[:])
            idb_sb = cpool.tile([TP, TP], bf16, tag="idb")
            nc.sync.dma_start(idb_sb[:], identb[:])
            iota_sb = cpool.tile([TP, TP], bf16, tag="io")
            nc.sync.dma_start(iota_sb[:], iota_in[:])
            dinv_sb = cpool.tile([TP, NT], f32, tag="dv")
            nc.sync.dma_start(dinv_sb[:], dinv_in[:])
            gidx_sb = cpool.tile([128, icols], mybir.dt.int16, tag="gi")
            nc.sync.dma_start(gidx_sb[:], gidx_in[:])
            col_sb = cpool.tile([TP, TB], bf16, tag="cl")
            nc.sync.dma_start(col_sb[:], col_in[:])

            # zero the tail rows of bounce once (rows 6272..6303)
            ztail = cpool.tile([32, E_F], bf16, tag="zt")
            nc.vector.memset(ztail[:], 0.0)
            nc.sync.dma_start(bounce[NT * TP :, :], ztail[:])

            # persistent activations: node-major [128, NT*128]
            z_sb = zpool.tile([TP, NT * E_F], bf16, tag="z")
            agg_sb = zpool.tile([TP, NT * E_F], f32, tag="agg")
            hp_keep = zpool.tile([TP, NT * E_F], bf16, tag="hpk")

            # layer 1 input: act_sb = x (dinv folded into the diag transpose)
            for t in range(NT):
                nc.gpsimd.dma_start(
                    z_sb[:, t * E_F : (t + 1) * E_F], x_in[t * TP : (t + 1) * TP, :]
                )


            # h' = z @ W -> hp_keep (SBUF) + bounce (DRAM)
            def transform(layer, t):
                zt_ps = pst.tile([TP, TP], f32, tag="tr")
                nc.tensor.matmul(
                    zt_ps[:],
                    z_sb[:, t * E_F : (t + 1) * E_F],
                    dg_sb[:, t * TP : (t + 1) * TP],
                    start=True,
                    stop=True,
                )
                zt = wpool.tile([TP, TP], f32, tag="ztr")
                nc.scalar.copy(zt[:], zt_ps[:])
                hp_ps = psh.tile([TP, E_F], f32, tag="hp")
                nc.tensor.matmul(
                    hp_ps[:], zt[:], w_sb[layer][:], start=True, stop=True
                )
                dstc = hp_keep[:, t * E_F : (t + 1) * E_F]
                nc.scalar.copy(dstc, hp_ps[:])
                nc.sync.dma_start(bounce[t * TP : (t + 1) * TP, :], dstc)

            def issue_ag(layer, g):
                a, b = GRPS[g]
                nc.gpsimd.collective_compute(
                    "AllGather",
                    mybir.AluOpType.bypass,
                    replica_groups=[list(range(CORES))],
                    ins=[bounce[a:b, :].opt()],
                    outs=[tables[layer][g][:].opt()],
                )

            # final stage: y[t] = sigmoid(act3 @ Wout + bout)
            y_sb = cpool.tile([TP, NT], f32, tag="y")

            def final_tile(t):
                zt_ps = pst.tile([TP, TP], bf16, tag="tr")
                nc.tensor.transpose(
                    zt_ps[:], z_sb[:, t * E_F : (t + 1) * E_F], idb_sb[:]
                )
                zt = wpool.tile([TP, TP], bf16, tag="ztr")
                nc.scalar.copy(zt[:], zt_ps[:])
                o_ps = psh.tile([TP, 1], f32, tag="hp")
                nc.tensor.matmul(o_ps[:], zt[:], woutb_sb[:], start=True, stop=True)
                nc.scalar.activation(
                    y_sb[:, t : t + 1],
                    o_ps[:],
                    mybir.ActivationFunctionType.Sigmoid,
                    bias=bout_sb[:],
                )

            # persistent num_idxs registers: one per distinct chunk size so
            # gathers share read-only regs (no per-gather MOVE -> no WAR
            # serialization on the GpSimd SEQ)
            nidx_regs = {}
            for _, _, nb in meta["chunks"]:
                if nb not in nidx_regs:
                    r = nc.alloc_register(mybir.EngineType.Pool, f"nidx{nb}")
                    nc.gpsimd.reg_mov(r, nb * TP)
                    nidx_regs[nb] = r

            # layer-0 transforms from x; each AG fires as soon as its
            # source tiles are written
            GRP_LAST_TILE = [min((b - 1) // TP, NT - 1) for _, b in GRPS]  # [19, 34, 48]
            for t in range(NT):
                transform(0, t)
                for g, lt in enumerate(GRP_LAST_TILE):
                    if t == lt:
                        issue_ag(0, g)

            for layer in range(3):

                # gather + segmented sum; next layer's transform + AGs are
                # emitted inline as each tile's aggregation finalizes
                cur_ps = None
                blocks_done = np.zeros(NT, np.int64)
                blocks_tot = {
                    (t, h): int(meta["blocks"][t, h])
                    for t in range(NT)
                    for h in range(NG)
                }
                cum_tot = {
                    (t, h): sum(int(meta["blocks"][t, g]) for g in range(h + 1))
                    for t in range(NT)
                    for h in range(NG)
                }
                for ci, (h, b0, nb) in enumerate(meta["chunks"]):
                    msg = mpool.tile([TP, CHB, E_F], bf16, tag="m")
                    src_ap = tables[layer][h][:, :]
                    nc.gpsimd.dma_gather(
                        msg[:, :nb, :],
                        src_ap,
                        gidx_sb[:, b0 * 8 : (b0 + nb) * 8],
                        nb * TP,
                        nidx_regs[nb],
                        E_F,
                        single_packet=True,
                        queue_num=ci % 4,
                    )
                    # S matrices built on-chip in groups of SGRP blocks
                    for g0 in range(0, nb, SGRP):
                        gn = min(SGRP, nb - g0)
                        s_sb = spool.tile([TP, SGRP, TP], bf16, tag="s")
                        nc.vector.tensor_tensor(
                            s_sb[:, :gn, :],
                            iota_sb[:].unsqueeze(1).broadcast_to([TP, gn, TP]),
                            col_sb[:, b0 + g0 : b0 + g0 + gn]
                            .unsqueeze(2)
                            .broadcast_to([TP, gn, TP]),
                            mybir.AluOpType.is_equal,
                        )
                        for i in range(gn):
                            gb = b0 + g0 + i
                            t, hh = meta["sched"][gb]
                            # per-group progress bookkeeping
                            prev_tot = cum_tot[(t, hh)] - blocks_tot[(t, hh)]
                            done_h = blocks_done[t] - prev_tot
                            first = done_h == 0
                            last = done_h + 1 == blocks_tot[(t, hh)]
                            if first:
                                cur_ps = pss.tile([TP, E_F], f32, tag="sa")
                                if hh == 0:
                                    # self-loop contribution: hp row d for dst d
                                    nc.tensor.matmul(
                                        cur_ps[:],
                                        idb_sb[:],
                                        hp_keep[:, t * E_F : (t + 1) * E_F],
                                        start=True,
                                        stop=False,
                                    )
                                else:
                                    # re-inject running partial from agg_sb
                                    nc.tensor.matmul(
                                        cur_ps[:],
                                        id_sb[:],
                                        agg_sb[:, t * E_F : (t + 1) * E_F],
                                        start=True,
                                        stop=False,
                                    )
                            nc.tensor.matmul(
                                cur_ps[:],
                                s_sb[:, i, :],
                                msg[:, g0 + i, :],
                                start=False,
                                stop=(last and hh < NG - 1),
                            )
                            blocks_done[t] += 1
                            if last and hh < NG - 1:
                                dstc = agg_sb[:, t * E_F : (t + 1) * E_F]
                                nc.scalar.activation(
                                    dstc, cur_ps[:], mybir.ActivationFunctionType.Copy
                                )
                            elif last:
                                # fold in the (1/dinv)*b bias
                                nc.tensor.matmul(
                                    cur_ps[:],
                                    c_sb[:, t * TP : (t + 1) * TP],
                                    b_sb[layer][:],
                                    start=False,
                                    stop=True,
                                )
                                # act = relu(dinv * agg_total) straight from PSUM
                                nc.scalar.activation(
                                    z_sb[:, t * E_F : (t + 1) * E_F],
                                    cur_ps[:],
                                    mybir.ActivationFunctionType.Relu,
                                    scale=dinv_sb[:, t : t + 1],
                                )
                                # tile t's activation is final: feed the next
                                # stage now so AGs overlap the gather phase
                                if layer < 2:
                                    transform(layer + 1, t)
                                    for g, lt in enumerate(GRP_LAST_TILE):
                                        if t == lt:
                                            issue_ag(layer + 1, g)
                                else:
                                    final_tile(t)

            nc.sync.dma_start(y_out[:], y_sb[:])

    nc.compile()
    return nc


def kernel(x, edge_index, W1, b1, W2, b2, W3, b3, Wout, bout):
    global _compiled
    from concourse.bass_utils import run_bass_kernel_spmd
    import ml_dtypes

    x = np.asarray(x, np.float32)
    dst = np.asarray(edge_index[1]).astype(np.int64)
    deg = np.bincount(dst, minlength=N).astype(np.float32) + 1.0
    dinv = (1.0 / np.sqrt(deg)).astype(np.float32)

    meta, per_core = _preprocess(edge_index)
    if _compiled is None:
        _compiled = _build(meta)
    nc = _compiled

    ident = np.eye(128, dtype=np.float32)
    identb = np.eye(128, dtype=ml_dtypes.bfloat16)
    iota = np.broadcast_to(
        np.arange(128, dtype=np.float32), (128, 128)
    ).astype(ml_dtypes.bfloat16)
    in_maps = []
    for p in range(CORES):
        lo = p * NP
        xp = np.zeros((NT * TP, E_F), np.float32)
        xp[:NP] = x[lo : lo + NP]
        dv = np.zeros(NT * TP, np.float32)
        dv[:NP] = dinv[lo : lo + NP]
        dinv_arr = dv.reshape(NT, TP).T.copy()  # [128, NT]
        dg = np.zeros((TP, NT * TP), ml_dtypes.bfloat16)
        for t in range(NT):
            dg[np.arange(TP), t * TP + np.arange(TP)] = dv[t * TP : (t + 1) * TP]
        cvec = np.zeros((1, NT * TP), np.float32)
        cvec[0, :NP] = np.sqrt(deg[lo : lo + NP])
        in_maps.append(
            {
                "x_in": xp,
                "dinv_in": dinv_arr,
                "W1": np.asarray(W1, np.float32),
                "W2": np.asarray(W2, np.float32),
                "W3": np.asarray(W3, np.float32),
                "b1": np.asarray(b1, np.float32).reshape(1, E_F),
                "b2": np.asarray(b2, np.float32).reshape(1, E_F),
                "b3": np.asarray(b3, np.float32).reshape(1, E_F),
                "dg_in": dg,
                "c_in": cvec,
                "Wout": np.asarray(Wout, np.float32).reshape(E_F, 1),
                "Woutb": np.asarray(Wout, np.float32).reshape(E_F, 1).astype(ml_dtypes.bfloat16),
                "bout": np.full((TP, 1), np.asarray(bout, np.float32).reshape(-1)[0], np.float32),
                "ident": ident,
                "identb": identb,
                "iota_in": iota,
                "gidx_in": per_core[p]["gidx"],
                "col_in": per_core[p]["col"],
            }
        )

    global _last_in_maps
    _last_in_maps = in_maps
    res = run_bass_kernel_spmd(nc, in_maps, core_ids=list(range(CORES)))
    out = np.zeros((N, 1), np.float32)
    for p in range(CORES):
        y = res.results[p]["y_out"]  # [128, NT]
        out[p * NP : (p + 1) * NP, 0] = y.T.reshape(-1)[:NP]
    return out
